# revision 3
# baseline (speedup 1.0000x reference)
"""Trainium2 Bass kernel for the PerceptualEntropy loss (segment_reduce).

Strategy
--------
Data parallel over the flattened (B*T) row dim: 38400 rows of 512 freq
bins, split as 4800 rows/core across 8 cores (padded with 64 zero rows
to 4864 = 38 blocks of 128; zero rows contribute exactly 0 to the loss).

Math (algebraically identical to the reference, up to fp rounding):
  the reference's `toff` cancels between `t` and `renorm`, and
  10^(log10(x)) == x, so
      t       = (psum @ S + eps) / (colsum(S) + eps)      [band level]
      2/denom = min(cf_f * rsqrt(t[seg f]), qmax_f)
  with cf_f = 2*sqrt(k_f/6), qmax_f = cf_f/sqrt(TQ_f) (TQ_f > 0 only).
  Loss terms log2(|x|*spec*q + 1) are computed as ln(.)/ln2 with the
  1/ln2 folded into the final host-side weights.

Device pipeline per 512-row macro tile (f chunks of 128, T = transposed
[f, row] layout produced by PE transposes):
  DMA   : natural [128, G*512] loads of lm/real/imag
  PE    : transpose lm -> ACT exp -> specT ; band matmul (MASK) ;
          spread matmul (S) ; expand matmul (scaled MASK^T) ;
          transpose real/imag
  ACT   : exp, band ops (ln, exp), ln(h+1) with fused accum_out row sums
  DVE   : m = (qpre min qmax) * specT ; h = |xT| * m   (scalar_tensor_tensor)
Final scalar assembled on host from per-core [128, 80] accumulators.
"""

import math

import numpy as np

# ---------------- problem constants (hardcoded) ----------------
WIDTHS = np.array(
    [3, 3, 3, 3, 4, 4, 4, 5, 5, 6, 7, 8, 9, 10, 12, 14, 18, 22, 29, 35, 42, 58, 80, 128],
    dtype=np.int64,
)
NB = 24
F = 512
FS = 32000
WIN = 1024
B, T = 32, 1200
EPS = 1e-8
N_CORES = 8

SEG = np.repeat(np.arange(NB), WIDTHS)  # [F] bin -> band
MASK = np.zeros((F, NB), dtype=np.float32)
MASK[np.arange(F), SEG] = 1.0
KB = WIDTHS.astype(np.float32)
_f = (np.arange(F) + 1.0) * FS / WIN
TQ = (
    3.64 * (_f / 1000 + 1e-6) ** (-0.8)
    - 6.5 * np.exp(-0.6 * (_f / 1000 - 3.3) ** 2)
    + 1e-3 * (_f / 1000) ** 4
).astype(np.float32)
KBIN = KB[SEG]  # [F]
CF = (2.0 * np.sqrt(KBIN / 6.0)).astype(np.float32)  # [F] = 2*sqrt(k/6)

ROWS = B * T            # 38400
RPC = ROWS // N_CORES   # 4800 real rows per core
RPAD = 4864             # 38 blocks of 128
MACROS = [4] * 9 + [2]  # row-blocks per macro tile (38 blocks total)
NMAC = len(MACROS)
NCHUNK = 4              # 512 bins = 4 chunks of 128
ACC_COLS = NMAC * NCHUNK * 2

_PROGRAM_CACHE = {}


def _build_program():
    """Build + compile the (single, shared-by-all-cores) Bass program."""
    import concourse.bass as bass  # noqa: F401
    import concourse.tile as tile
    from concourse import bacc, mybir
    from contextlib import ExitStack

    dt = mybir.dt.float32
    AF = mybir.ActivationFunctionType
    OP = mybir.AluOpType

    nc = bacc.Bacc("TRN2", target_bir_lowering=False, debug=False, enable_asserts=False)

    lm_d = nc.dram_tensor("lm", [RPAD, F], dt, kind="ExternalInput").ap()
    re_d = nc.dram_tensor("re", [RPAD, F], dt, kind="ExternalInput").ap()
    im_d = nc.dram_tensor("im", [RPAD, F], dt, kind="ExternalInput").ap()
    sp_d = nc.dram_tensor("spread", [NB, NB], dt, kind="ExternalInput").ap()
    ident_d = nc.dram_tensor("ident", [128, 128], dt, kind="ExternalInput").ap()
    maskt_d = nc.dram_tensor("maskt", [128, NCHUNK * NB], dt, kind="ExternalInput").ap()
    emask_d = nc.dram_tensor("emask", [NB, F], dt, kind="ExternalInput").ap()
    ppack_d = nc.dram_tensor("ppack", [128, 6], dt, kind="ExternalInput").ap()
    acc_d = nc.dram_tensor("acc", [128, ACC_COLS], dt, kind="ExternalOutput").ap()

    with tile.TileContext(nc) as tc, ExitStack() as ctx:
        cpool = ctx.enter_context(tc.tile_pool(name="consts", bufs=1))
        lmpool = ctx.enter_context(tc.tile_pool(name="lmN", bufs=2))
        repool = ctx.enter_context(tc.tile_pool(name="reN", bufs=2))
        impool = ctx.enter_context(tc.tile_pool(name="imN", bufs=2))
        stpool = ctx.enter_context(tc.tile_pool(name="specT", bufs=2))
        smpool = ctx.enter_context(tc.tile_pool(name="small", bufs=2))
        mpool = ctx.enter_context(tc.tile_pool(name="m", bufs=3))
        hpool = ctx.enter_context(tc.tile_pool(name="h", bufs=3))
        pepool = ctx.enter_context(tc.tile_pool(name="pe", bufs=2))
        # PSUM: 8 banks total; budget = lmT(2) + band(1) + SP(1) + qpre(2) + xT(2)
        lmtpool = ctx.enter_context(tc.tile_pool(name="lmT", bufs=2, space="PSUM"))
        bandpool = ctx.enter_context(tc.tile_pool(name="band", bufs=1, space="PSUM"))
        sppool = ctx.enter_context(tc.tile_pool(name="SP", bufs=1, space="PSUM"))
        qpool = ctx.enter_context(tc.tile_pool(name="qpre", bufs=2, space="PSUM"))
        xtpool = ctx.enter_context(tc.tile_pool(name="xT", bufs=2, space="PSUM"))

        ident_s = cpool.tile([128, 128], dt)
        nc.sync.dma_start(ident_s[:], ident_d[:])
        maskt_s = cpool.tile([128, NCHUNK * NB], dt)
        nc.sync.dma_start(maskt_s[:], maskt_d[:])
        emask_s = cpool.tile([NB, F], dt)
        nc.sync.dma_start(emask_s[:], emask_d[:])
        ppack_s = cpool.tile([128, 6], dt)
        nc.sync.dma_start(ppack_s[:], ppack_d[:])
        spread_s = cpool.tile([NB, NB], dt)
        nc.sync.dma_start(spread_s[:], sp_d[:])
        acc_s = cpool.tile([128, ACC_COLS], dt)

        row0 = 0
        for mi, G in enumerate(MACROS):
            R = G * 128

            def load(pool, src):
                t = pool.tile([128, G * F], dt)
                sap = src[row0 : row0 + G * 128, :].rearrange("(b p) f -> p b f", p=128)
                nc.sync.dma_start(t[:].rearrange("p (b f) -> p b f", f=F), sap)
                return t

            lmN = load(lmpool, lm_d)
            reN = load(repool, re_d)
            imN = load(impool, im_d)

            specT = stpool.tile([128, NCHUNK * R], dt)
            band = bandpool.tile([NB, R], dt)
            for c in range(NCHUNK):
                lmT = lmtpool.tile([128, R], dt)
                for b in range(G):
                    nc.tensor.transpose(
                        lmT[:, b * 128 : (b + 1) * 128],
                        lmN[:, b * F + c * 128 : b * F + (c + 1) * 128],
                        ident_s[:],
                    )
                nc.scalar.activation(specT[:, c * R : (c + 1) * R], lmT[:], AF.Exp)
                nc.tensor.matmul(
                    band[:],
                    maskt_s[:, c * NB : (c + 1) * NB],
                    specT[:, c * R : (c + 1) * R],
                    start=(c == 0),
                    stop=(c == NCHUNK - 1),
                )

            pband = smpool.tile([NB, R], dt, tag="pband")
            nc.scalar.copy(pband[:], band[:])
            SPp = sppool.tile([NB, R], dt)
            nc.tensor.matmul(SPp[:], spread_s[:], pband[:], start=True, stop=True)
            # l1 = ln(rcoef*SP + eps*rcoef) ; rt = exp(-l1/2) = rsqrt(t)
            l1 = smpool.tile([NB, R], dt, tag="l1")
            nc.scalar.activation(
                l1[:], SPp[:], AF.Ln,
                bias=ppack_s[0:NB, 5:6], scale=ppack_s[0:NB, 4:5],
            )
            rt = smpool.tile([NB, R], dt, tag="rt")
            nc.scalar.activation(rt[:], l1[:], AF.Exp, scale=-0.5)

            for c in range(NCHUNK):
                qpre = qpool.tile([128, R], dt)
                nc.tensor.matmul(
                    qpre[:], emask_s[:, c * 128 : (c + 1) * 128], rt[:],
                    start=True, stop=True,
                )
                mt = mpool.tile([128, R], dt)
                nc.vector.scalar_tensor_tensor(
                    mt[:], qpre[:], ppack_s[:, c : c + 1],
                    specT[:, c * R : (c + 1) * R],
                    OP.min, OP.mult,
                )
                for ti, xN in enumerate((reN, imN)):
                    xT = xtpool.tile([128, R], dt)
                    for b in range(G):
                        nc.tensor.transpose(
                            xT[:, b * 128 : (b + 1) * 128],
                            xN[:, b * F + c * 128 : b * F + (c + 1) * 128],
                            ident_s[:],
                        )
                    ht = hpool.tile([128, R], dt)
                    # inputs are |real|/|imag| (abs taken host-side), so a
                    # plain multiply suffices here
                    nc.vector.tensor_tensor(ht[:], xT[:], mt[:], OP.mult)
                    pe = pepool.tile([128, R], dt)
                    col = (mi * NCHUNK + c) * 2 + ti
                    nc.scalar.activation(
                        pe[:], ht[:], AF.Ln, bias=1.0,
                        accum_out=acc_s[:, col : col + 1],
                    )
            row0 += G * 128

        nc.sync.dma_start(acc_d[:], acc_s[:])

    nc.compile()
    return nc


def get_program():
    if "nc" not in _PROGRAM_CACHE:
        _PROGRAM_CACHE["nc"] = _build_program()
    return _PROGRAM_CACHE["nc"]


def make_in_maps(log_magnitude, real, imag, spread_function):
    lm = np.ascontiguousarray(np.asarray(log_magnitude, np.float32).reshape(ROWS, F))
    # only |real| / |imag| are ever used (|real*spec| = |real|*spec), so take
    # the abs on the host and skip it on-device entirely
    re = np.abs(np.asarray(real, np.float32).reshape(ROWS, F))
    im = np.abs(np.asarray(imag, np.float32).reshape(ROWS, F))
    S = np.ascontiguousarray(np.asarray(spread_function, np.float32))

    cs = S.sum(axis=0)
    rcoef = (1.0 / (cs + EPS)).astype(np.float32)
    ppack = np.zeros((128, 6), np.float32)
    qmax = np.where(TQ > 0, CF / np.sqrt(np.maximum(TQ, 1e-30)), 3e38).astype(np.float32)
    ppack[:, 0:4] = qmax.reshape(4, 128).T
    ppack[0:NB, 4] = rcoef
    ppack[0:NB, 5] = np.float32(EPS) * rcoef

    maskt = np.zeros((128, NCHUNK * NB), np.float32)
    emask = np.zeros((NB, F), np.float32)
    for c in range(NCHUNK):
        mc = MASK[c * 128 : (c + 1) * 128, :]  # [128, 24]
        maskt[:, c * NB : (c + 1) * NB] = mc
        emask[:, c * 128 : (c + 1) * 128] = (mc * CF[c * 128 : (c + 1) * 128, None]).T

    ident = np.eye(128, dtype=np.float32)

    in_maps = []
    pad = np.zeros((RPAD - RPC, F), np.float32)
    for c in range(N_CORES):
        sl = slice(c * RPC, (c + 1) * RPC)
        in_maps.append(
            {
                "lm": np.concatenate([lm[sl], pad], axis=0),
                "re": np.concatenate([re[sl], pad], axis=0),
                "im": np.concatenate([im[sl], pad], axis=0),
                "spread": S,
                "ident": ident,
                "maskt": maskt,
                "emask": emask,
                "ppack": ppack,
            }
        )
    return in_maps


def finish(acc_stack):
    """acc_stack: [n_cores, 128, ACC_COLS] -> final scalar."""
    a = acc_stack.reshape(len(acc_stack), 128, NMAC, NCHUNK, 2)
    per_fc = a.sum(axis=(0, 2, 4), dtype=np.float64)  # [128, NCHUNK]
    f_idx = np.arange(NCHUNK)[None, :] * 128 + np.arange(128)[:, None]  # [128, NCHUNK]
    w = 1.0 / (B * T * KBIN[f_idx].astype(np.float64) * math.log(2.0))
    loss = float((per_fc * w).sum())
    return np.float32(1.0 / (loss + 1.0))


def kernel(log_magnitude, real, imag, spread_function):
    from concourse.bass_utils import run_bass_kernel_spmd

    nc = get_program()
    in_maps = make_in_maps(log_magnitude, real, imag, spread_function)
    res = run_bass_kernel_spmd(nc, in_maps, list(range(N_CORES))).results
    acc = np.stack([r["acc"] for r in res])
    return finish(acc)
